# revision 14
# baseline (speedup 1.0000x reference)
"""Self-contained Trainium2 Bass kernel for a single attention head.

Problem: B=8, S=2048, E=1024, D=64 (fp32 in/out).
  q = query @ Wq.T + bq ; k, v likewise
  out = softmax(mask(q @ k.T / sqrt(D))) @ v
  mask = query_mask[:, :, None] * key_mask[:, None, :]; query_mask is all-ones
  per the problem spec (fill="ones").

Sharding: pure data-parallel, one batch element per NeuronCore (8 cores).

Key ideas (v3):
  - fp16 compute with fp32 PSUM accumulation (rel err ~7e-4 vs f32 ref).
  - Host compacts away masked key columns; S_k shrinks 2048 -> ~1100,
    padded to a multiple of 128; pad columns get exp bias -30000 -> 0.
  - All input staging on the HWDGE SP ring (live ~4us before SWDGE) as
    fat contiguous pieces ordered by consumption deadline; the front
    pieces are 256-col (0.5MB) so the first matmul fires ~10us.
  - Scores contract K=64 directly (no zero-pad): matmul time only
    depends on the moving free dim and LDWEIGHTS hides behind matmuls.
  - Softmax denominator folds into the AV matmul as a 65th output row
    (ones row lives in the vT65 projection tile).
  - No on-chip normalize/transpose finale: raw [65, S] numerator rows
    go PSUM -> SBUF fp16 -> DRAM; the host does (num[:64]/num[64]).T.
    The tail copies/stores run split across ACT + DVE and the two HWDGE
    rings so the post-matmul tail is ~1.5us.
  - Emission interleaves projection pieces and score pairs so the PE
    never waits on DMA for long, and the exp chain (19.1us of ACT, the
    softmax floor) starts as early as the q half-0 + first key piece
    allow and is never starved after.
"""

from contextlib import ExitStack

import numpy as np

import concourse.bass as bass
import concourse.mybir as mybir
import concourse.tile as tile
from concourse import bacc
from concourse.bass_utils import run_bass_kernel_spmd
from concourse.masks import make_identity

FP16 = mybir.dt.float16
F32 = mybir.dt.float32

N_CORES = 8
B, S, E, D = 8, 2048, 1024, 64
P = 128
NE = E // P            # 8 contraction tiles
NH = 2                 # query halves (PSUM capacity)
HI = S // NH           # 1024 query positions per half
NC = 512               # matmul free-dim chunk (one PSUM bank of f32)
SCALE = 1.0 / np.sqrt(np.float32(D))
MASK_NEG = -30000.0

QPIECES = [(0, 512), (512, 512), (1024, 512), (1536, 512)]


def _chunks(total, step, base=0):
    out = []
    o = 0
    while o < total:
        out.append((base + o, min(step, total - o)))
        o += step
    return out


def _kpieces(sk2):
    return _chunks(sk2, NC)


def _build(tc: tile.TileContext, ins: dict, out_d: bass.AP, ctx, sk2: int):
    nc = tc.nc
    nj = sk2 // P
    kp = _kpieces(sk2)
    vp = _chunks(sk2, NC)
    pairs = [tuple(j for j in (j0, j0 + 1) if j < nj)
             for j0 in range(0, nj, 2)]

    consts = ctx.enter_context(tc.tile_pool(name="consts", bufs=1))
    stage = ctx.enter_context(tc.tile_pool(name="stage", bufs=1))
    proj = ctx.enter_context(tc.tile_pool(name="proj", bufs=1))
    xpool = ctx.enter_context(tc.tile_pool(name="xpool", bufs=max(nj, 2)))
    ppool = ctx.enter_context(tc.tile_pool(name="ppool", bufs=max(2 * nj, 2)))
    fin = ctx.enter_context(tc.tile_pool(name="fin", bufs=1))
    ps_mm = ctx.enter_context(tc.tile_pool(name="ps_mm", bufs=2, space="PSUM"))
    ps_sm = ctx.enter_context(tc.tile_pool(name="ps_sm", bufs=2, space="PSUM"))
    ps_acc = ctx.enter_context(tc.tile_pool(name="ps_acc", bufs=1,
                                            space="PSUM"))

    # --- staged inputs, HWDGE SP ring, consumption-deadline order --------
    c16 = consts.tile([P, 3 * NE * D], FP16, tag="c16")
    c32 = consts.tile([P, nj + 3], F32, tag="c32")
    qs = [stage.tile([P, NE * n], FP16, tag=f"q{i}", name=f"qs{i}")
          for i, (o, n) in enumerate(QPIECES)]
    ks = [stage.tile([P, NE * n], FP16, tag=f"k{i}", name=f"ks{i}")
          for i, (o, n) in enumerate(kp)]
    vs = [stage.tile([P, NE * n], FP16, tag=f"v{i}", name=f"vs{i}")
          for i, (o, n) in enumerate(vp)]

    nc.sync.dma_start(out=c16[:], in_=ins["c16"][:])
    nc.sync.dma_start(out=c32[:], in_=ins["c32"][:])
    nc.sync.dma_start(out=qs[0][:], in_=ins["q0"][:])
    nc.sync.dma_start(out=ks[0][:], in_=ins["k0"][:])
    nc.sync.dma_start(out=qs[1][:], in_=ins["q1"][:])
    if len(ks) > 1:
        nc.sync.dma_start(out=ks[1][:], in_=ins["k1"][:])
    for i in range(2, len(ks)):
        nc.sync.dma_start(out=ks[i][:], in_=ins[f"k{i}"][:])
    nc.sync.dma_start(out=qs[2][:], in_=ins["q2"][:])
    nc.sync.dma_start(out=qs[3][:], in_=ins["q3"][:])
    for i in range(len(vs)):
        nc.sync.dma_start(out=vs[i][:], in_=ins[f"v{i}"][:])

    wq = c16[:, 0:NE * D]
    wk = c16[:, NE * D:2 * NE * D]
    wv = c16[:, 2 * NE * D:3 * NE * D]
    mb = c32[:, 0:nj]
    bq = c32[0:D, nj:nj + 1]
    bk = c32[0:D, nj + 1:nj + 2]
    bv = c32[0:D, nj + 2:nj + 3]

    # --- engine warm-up / constants --------------------------------------
    ident = consts.tile([P, P], FP16, tag="ident")
    warm = consts.tile([P, 16], F32, tag="warm")
    make_identity(nc, ident[:])
    nc.vector.memset(warm[:], 0.0)
    nc.scalar.activation(warm[:], warm[:], mybir.ActivationFunctionType.Exp)

    # persistent projected tensors
    qT = proj.tile([D, S], FP16, tag="qT")
    kT = proj.tile([D, sk2], FP16, tag="kT")
    vT65 = proj.tile([D + 1, sk2], FP16, tag="vT65")
    nc.vector.memset(vT65[D:D + 1, :], 1.0)   # ones row -> softmax denom

    # ---- projection helpers ---------------------------------------------
    def proj_piece(dst, w, bias_ap, src, o, n):
        ps = ps_sm.tile([D, NC], F32, tag="ps_sm",
                        name=f"ps_{dst.tensor.name}_{o}")
        for e in range(NE):
            nc.tensor.matmul(
                ps[0:D, 0:n],
                w[:, e * D:(e + 1) * D],
                src[:, e * n:e * n + n],
                start=(e == 0), stop=(e == NE - 1),
            )
        nc.vector.tensor_scalar_add(dst[0:D, o:o + n], ps[0:D, 0:n], bias_ap)

    def proj_items(dst, w, bias_ap, src, o, n):
        """Two ~0.9us pump items (4 e-passes each; 2nd emits bias add)."""
        st = {}

        def sub(eh):
            if eh == 0:
                st["ps"] = ps_sm.tile([D, NC], F32, tag="ps_sm",
                                      name=f"psp_{dst.tensor.name}_{o}")
            ps = st["ps"]
            for e in range(eh * (NE // 2), (eh + 1) * (NE // 2)):
                nc.tensor.matmul(
                    ps[0:D, 0:n],
                    w[:, e * D:(e + 1) * D],
                    src[:, e * n:e * n + n],
                    start=(e == 0), stop=(e == NE - 1),
                )
            if eh == 1:
                nc.vector.tensor_scalar_add(
                    dst[0:D, o:o + n], ps[0:D, 0:n], bias_ap)

        return [lambda: sub(0), lambda: sub(1)]

    # ---- attention helpers ----------------------------------------------
    sst = {}
    pms = {}

    def spair(h, pr, cs=(0, 1)):
        for c in cs:
            for j in pr:
                if (h, j) not in sst:
                    sst[(h, j)] = ps_mm.tile([P, HI], F32, tag="ps_mm",
                                             name=f"ssT_{h}_{j}")
                nc.tensor.matmul(
                    sst[(h, j)][:, c * NC:(c + 1) * NC],
                    kT[:, j * P:(j + 1) * P],
                    qT[:, h * HI + c * NC:h * HI + (c + 1) * NC],
                    start=True, stop=True,
                )
        if 1 in cs:
            for j in pr:
                p = ppool.tile([P, HI], FP16, tag="pm", name=f"pm_{h}_{j}")
                nc.scalar.activation(p[:], sst[(h, j)][:],
                                     mybir.ActivationFunctionType.Exp,
                                     bias=mb[:, j:j + 1], scale=float(SCALE))
                pms[(h, j)] = p

    def sone(h, j, cs=(0, 1)):
        spair(h, (j,), cs=cs)

    xt = [None] * nj

    def x_group(js):
        for j in js:
            pst = ps_sm.tile([P, D + 1], FP16, tag="ps_sm", name=f"psx{j}")
            nc.tensor.transpose(pst[:], vT65[:, j * P:(j + 1) * P],
                                ident[0:D + 1, 0:D + 1])
            x = xpool.tile([P, D + 1], FP16, tag="x", name=f"x{j}")
            nc.vector.tensor_copy(x[:], pst[:])
            xt[j] = x

    def av_h0(num0, js):
        for j in js:
            for c in range(HI // NC):
                nc.tensor.matmul(
                    num0[:, c * NC:(c + 1) * NC],
                    xt[j][:],
                    pms[(0, j)][:, c * NC:(c + 1) * NC],
                    start=(j == 0), stop=(j == nj - 1),
                )

    # ---- emission --------------------------------------------------------
    def pq(i):
        o, n = QPIECES[i]
        proj_piece(qT, wq, bq, qs[i][:], o, n)

    def pk(i):
        o, n = kp[i]
        proj_piece(kT, wk, bk, ks[i][:], o, n)

    # num0 is allocated first so warm-up matmuls can target its PSUM; the
    # real AV h0 accumulation later resets it via start=True.
    num0 = ps_acc.tile([D + 1, HI], F32, tag="num", name="num0")

    # front: q half0 chunk0 + first key piece -> first score chunks;
    # q1 -> first exps; then the h0 per-j chain with k/q pieces between
    pq(0)
    pk(0)
    sone(0, 0, cs=(0,))
    if nj > 1:
        sone(0, 1, cs=(0,))
    pq(1)
    sone(0, 0, cs=(1,))
    if nj > 1:
        sone(0, 1, cs=(1,))
    for j in range(2, min(4, nj)):
        sone(0, j)
    if len(kp) > 1:
        pk(1)
    for j in range(4, min(6, nj)):
        sone(0, j)
    pq(2)
    for j in range(6, min(8, nj)):
        sone(0, j)
    for i in range(2, len(kp)):
        pk(i)
    pq(3)
    for j in range(8, nj):
        sone(0, j)

    # h1 scores per-j; v half-piece items chase their DMA pieces; the
    # x transposes / AV h0 / most of AV h1 are emitted BEFORE the last
    # h1 score matmuls (whose ssT slots wait deep into the exp chain) so
    # the AV work overlaps the exp tail instead of serializing after it.
    fillv = []
    for i, (o, n) in enumerate(vp):
        fillv += proj_items(vT65, wv, bv, vs[i][:], o, n)
    for j in range(nj - 2):
        sone(1, j)
        if j >= 3 and fillv:
            fillv.pop(0)()
    while fillv:
        fillv.pop(0)()

    jsets = [list(range(o // P, (o + n) // P)) for (o, n) in vp]
    for js in jsets:
        x_group(js)
        av_h0(num0, js)
    nsb0 = fin.tile([D + 1, HI], FP16, tag="nsb0")
    nc.vector.tensor_copy(nsb0[:], num0[:])
    nc.sync.dma_start(out=out_d[0:D + 1, :], in_=nsb0[:])

    for j in range(max(nj - 2, 0), nj):
        sone(1, j)

    numc = [ps_sm.tile([D + 1, NC], F32, tag="ps_sm", name=f"num1c{c}")
            for c in range(HI // NC)]
    nsb1 = [fin.tile([D + 1, NC], FP16, tag=f"nsb1{c}", name=f"nsb1{c}")
            for c in range(HI // NC)]
    for j in range(nj):
        for c in range(HI // NC):
            nc.tensor.matmul(
                numc[c][:],
                xt[j][:],
                pms[(1, j)][:, c * NC:(c + 1) * NC],
                start=(j == 0), stop=(j == nj - 1),
            )
            if j == nj - 1:
                if c == 0:
                    nc.scalar.activation(
                        nsb1[c][:], numc[c][:],
                        mybir.ActivationFunctionType.Copy)
                else:
                    nc.vector.tensor_copy(nsb1[c][:], numc[c][:])
                nc.sync.dma_start(
                    out=out_d[D + 1:2 * (D + 1), c * NC:(c + 1) * NC],
                    in_=nsb1[c][:])


_COMPILED = {}


def _get_compiled(sk2: int):
    if sk2 not in _COMPILED:
        nj = sk2 // P
        kp = _kpieces(sk2)
        vp = _chunks(sk2, NC)
        nc = bacc.Bacc("TRN2", target_bir_lowering=False, debug=False,
                       num_devices=N_CORES)

        def din(name, shape, dt=FP16):
            return nc.dram_tensor(name, shape, dt, kind="ExternalInput").ap()

        ins = {"c16": din("c16", [P, 3 * NE * D]),
               "c32": din("c32", [P, nj + 3], F32)}
        for i, (o, n) in enumerate(QPIECES):
            ins[f"q{i}"] = din(f"q{i}", [P, NE * n])
        for i, (o, n) in enumerate(kp):
            ins[f"k{i}"] = din(f"k{i}", [P, NE * n])
        for i, (o, n) in enumerate(vp):
            ins[f"v{i}"] = din(f"v{i}", [P, NE * n])
        out_d = nc.dram_tensor("out", [NH * (D + 1), HI], FP16,
                               kind="ExternalOutput").ap()
        with tile.TileContext(nc) as tc:
            with ExitStack() as ctx:
                _build(tc, ins, out_d, ctx, sk2)
        nc.compile()
        _COMPILED[sk2] = nc
    return _COMPILED[sk2]


def _blob(x16, lo, hi):
    """[S', E] fp16 row-slice -> staging blob [P, NE*(hi-lo)] laid out as
    [partition, e-block, col]."""
    return np.ascontiguousarray(
        x16[lo:hi].reshape(hi - lo, NE, P).transpose(2, 1, 0)
    ).reshape(P, -1)


LAST_RESULTS = None


def kernel(query, key, value, query_mask, key_mask, Wq, bq, Wk, bk, Wv, bv):
    global LAST_RESULTS
    query = np.asarray(query, dtype=np.float32)
    key = np.asarray(key, dtype=np.float32)
    value = np.asarray(value, dtype=np.float32)
    key_mask = np.asarray(key_mask)

    # compact masked keys away (they contribute exactly zero)
    keeps = [np.nonzero(key_mask[c] != 0)[0] for c in range(N_CORES)]
    nk_max = max(len(kps) for kps in keeps)
    sk2 = max(P, int(np.ceil(nk_max / P)) * P)
    sk2 = min(sk2, S)
    nj = sk2 // P
    kp = _kpieces(sk2)
    vp = _chunks(sk2, NC)

    w16 = np.concatenate(
        [np.asarray(w, np.float32).astype(np.float16)
         .reshape(D, NE, P).transpose(2, 1, 0).reshape(P, NE * D)
         for w in (Wq, Wk, Wv)], axis=1)
    c32 = np.zeros((P, nj + 3), np.float32)
    for i, b in enumerate((bq, bk, bv)):
        c32[0:D, nj + i] = np.asarray(b, np.float32).reshape(D)

    in_maps = []
    for c in range(N_CORES):
        kps = keeps[c]
        nk = len(kps)
        q16 = query[c].astype(np.float16)
        kc = np.zeros((sk2, E), np.float16)
        vc = np.zeros((sk2, E), np.float16)
        kc[0:nk] = key[c][kps].astype(np.float16)
        vc[0:nk] = value[c][kps].astype(np.float16)
        c32c = c32.copy()
        mbias = np.full(sk2, np.float32(MASK_NEG))
        mbias[0:nk] = 0.0
        c32c[:, 0:nj] = mbias.reshape(nj, P).T
        im = {"c16": w16, "c32": np.ascontiguousarray(c32c)}
        for i, (o, n) in enumerate(QPIECES):
            im[f"q{i}"] = _blob(q16, o, o + n)
        for i, (o, n) in enumerate(kp):
            im[f"k{i}"] = _blob(kc, o, o + n)
        for i, (o, n) in enumerate(vp):
            im[f"v{i}"] = _blob(vc, o, o + n)
        in_maps.append(im)

    nc = _get_compiled(sk2)
    res = run_bass_kernel_spmd(nc, in_maps, core_ids=list(range(N_CORES)))
    LAST_RESULTS = res

    out = np.empty((N_CORES, S, D), np.float32)
    for c in range(N_CORES):
        o = np.asarray(res.results[c]["out"]).astype(np.float32)
        for h in range(NH):
            nh = o[h * (D + 1):(h + 1) * (D + 1)]
            out[c, h * HI:(h + 1) * HI] = (nh[0:D] / nh[D:D + 1]).T
    return out


# revision 15
# speedup vs baseline: 1.0187x; 1.0187x over previous
"""Self-contained Trainium2 Bass kernel for a single attention head.

Problem: B=8, S=2048, E=1024, D=64 (fp32 in/out).
  q = query @ Wq.T + bq ; k, v likewise
  out = softmax(mask(q @ k.T / sqrt(D))) @ v
  mask = query_mask[:, :, None] * key_mask[:, None, :]; query_mask is all-ones
  per the problem spec (fill="ones").

Sharding: pure data-parallel, one batch element per NeuronCore (8 cores).

Key ideas (v3):
  - fp16 compute with fp32 PSUM accumulation (rel err ~7e-4 vs f32 ref).
  - Host compacts away masked key columns; S_k shrinks 2048 -> ~1100,
    padded to a multiple of 128; pad columns get exp bias -30000 -> 0.
  - All input staging on the HWDGE SP ring (live ~4us before SWDGE) as
    fat contiguous pieces ordered by consumption deadline; the front
    pieces are 256-col (0.5MB) so the first matmul fires ~10us.
  - Scores contract K=64 directly (no zero-pad): matmul time only
    depends on the moving free dim and LDWEIGHTS hides behind matmuls.
  - Softmax denominator folds into the AV matmul as a 65th output row
    (ones row lives in the vT65 projection tile).
  - No on-chip normalize/transpose finale: raw [65, S] numerator rows
    go PSUM -> SBUF fp16 -> DRAM; the host does (num[:64]/num[64]).T.
    The tail copies/stores run split across ACT + DVE and the two HWDGE
    rings so the post-matmul tail is ~1.5us.
  - Emission interleaves projection pieces and score pairs so the PE
    never waits on DMA for long, and the exp chain (19.1us of ACT, the
    softmax floor) starts as early as the q half-0 + first key piece
    allow and is never starved after.
"""

from contextlib import ExitStack

import numpy as np

import concourse.bass as bass
import concourse.mybir as mybir
import concourse.tile as tile
from concourse import bacc
from concourse.bass_utils import run_bass_kernel_spmd
from concourse.masks import make_identity

FP16 = mybir.dt.float16
F32 = mybir.dt.float32

N_CORES = 8
B, S, E, D = 8, 2048, 1024, 64
P = 128
NE = E // P            # 8 contraction tiles
NH = 2                 # query halves (PSUM capacity)
HI = S // NH           # 1024 query positions per half
NC = 512               # matmul free-dim chunk (one PSUM bank of f32)
SCALE = 1.0 / np.sqrt(np.float32(D))
MASK_NEG = -30000.0

QPIECES = [(0, 512), (512, 512), (1024, 512), (1536, 512)]


def _chunks(total, step, base=0):
    out = []
    o = 0
    while o < total:
        out.append((base + o, min(step, total - o)))
        o += step
    return out


def _kpieces(sk2):
    return _chunks(sk2, NC)


def _build(tc: tile.TileContext, ins: dict, out_d: bass.AP, ctx, sk2: int):
    nc = tc.nc
    nj = sk2 // P
    kp = _kpieces(sk2)
    vp = _chunks(sk2, NC)
    pairs = [tuple(j for j in (j0, j0 + 1) if j < nj)
             for j0 in range(0, nj, 2)]

    consts = ctx.enter_context(tc.tile_pool(name="consts", bufs=1))
    stage = ctx.enter_context(tc.tile_pool(name="stage", bufs=1))
    proj = ctx.enter_context(tc.tile_pool(name="proj", bufs=1))
    xpool = ctx.enter_context(tc.tile_pool(name="xpool", bufs=max(nj, 2)))
    ppool = ctx.enter_context(tc.tile_pool(name="ppool", bufs=max(2 * nj, 2)))
    fin = ctx.enter_context(tc.tile_pool(name="fin", bufs=1))
    ps_mm = ctx.enter_context(tc.tile_pool(name="ps_mm", bufs=2, space="PSUM"))
    ps_sm = ctx.enter_context(tc.tile_pool(name="ps_sm", bufs=2, space="PSUM"))
    ps_acc = ctx.enter_context(tc.tile_pool(name="ps_acc", bufs=1,
                                            space="PSUM"))

    # --- staged inputs, HWDGE SP ring, consumption-deadline order --------
    c16 = consts.tile([P, 3 * NE * D], FP16, tag="c16")
    c32 = consts.tile([P, nj + 3], F32, tag="c32")
    qs = [stage.tile([P, NE * n], FP16, tag=f"q{i}", name=f"qs{i}")
          for i, (o, n) in enumerate(QPIECES)]
    ks = [stage.tile([P, NE * n], FP16, tag=f"k{i}", name=f"ks{i}")
          for i, (o, n) in enumerate(kp)]
    vs = [stage.tile([P, NE * n], FP16, tag=f"v{i}", name=f"vs{i}")
          for i, (o, n) in enumerate(vp)]

    nc.sync.dma_start(out=c16[:], in_=ins["c16"][:])
    nc.sync.dma_start(out=c32[:], in_=ins["c32"][:])
    nc.sync.dma_start(out=qs[0][:], in_=ins["q0"][:])
    nc.sync.dma_start(out=ks[0][:], in_=ins["k0"][:])
    nc.sync.dma_start(out=qs[1][:], in_=ins["q1"][:])
    if len(ks) > 1:
        nc.sync.dma_start(out=ks[1][:], in_=ins["k1"][:])
    for i in range(2, len(ks)):
        nc.sync.dma_start(out=ks[i][:], in_=ins[f"k{i}"][:])
    nc.sync.dma_start(out=qs[2][:], in_=ins["q2"][:])
    nc.sync.dma_start(out=qs[3][:], in_=ins["q3"][:])
    for i in range(len(vs)):
        nc.sync.dma_start(out=vs[i][:], in_=ins[f"v{i}"][:])

    wq = c16[:, 0:NE * D]
    wk = c16[:, NE * D:2 * NE * D]
    wv = c16[:, 2 * NE * D:3 * NE * D]
    mb = c32[:, 0:nj]
    bq = c32[0:D, nj:nj + 1]
    bk = c32[0:D, nj + 1:nj + 2]
    bv = c32[0:D, nj + 2:nj + 3]

    # --- engine warm-up / constants --------------------------------------
    ident = consts.tile([P, P], FP16, tag="ident")
    warm = consts.tile([P, 16], F32, tag="warm")
    make_identity(nc, ident[:])
    nc.vector.memset(warm[:], 0.0)
    nc.scalar.activation(warm[:], warm[:], mybir.ActivationFunctionType.Exp)

    # persistent projected tensors
    qT = proj.tile([D, S], FP16, tag="qT")
    kT = proj.tile([D, sk2], FP16, tag="kT")
    vT65 = proj.tile([D + 1, sk2], FP16, tag="vT65")
    nc.vector.memset(vT65[D:D + 1, :], 1.0)   # ones row -> softmax denom

    # ---- projection helpers ---------------------------------------------
    def proj_piece(dst, w, bias_ap, src, o, n):
        ps = ps_sm.tile([D, NC], F32, tag="ps_sm",
                        name=f"ps_{dst.tensor.name}_{o}")
        for e in range(NE):
            nc.tensor.matmul(
                ps[0:D, 0:n],
                w[:, e * D:(e + 1) * D],
                src[:, e * n:e * n + n],
                start=(e == 0), stop=(e == NE - 1),
            )
        nc.vector.tensor_scalar_add(dst[0:D, o:o + n], ps[0:D, 0:n], bias_ap)

    def proj_items(dst, w, bias_ap, src, o, n):
        """Two ~0.9us pump items (4 e-passes each; 2nd emits bias add)."""
        st = {}

        def sub(eh):
            if eh == 0:
                st["ps"] = ps_sm.tile([D, NC], F32, tag="ps_sm",
                                      name=f"psp_{dst.tensor.name}_{o}")
            ps = st["ps"]
            for e in range(eh * (NE // 2), (eh + 1) * (NE // 2)):
                nc.tensor.matmul(
                    ps[0:D, 0:n],
                    w[:, e * D:(e + 1) * D],
                    src[:, e * n:e * n + n],
                    start=(e == 0), stop=(e == NE - 1),
                )
            if eh == 1:
                nc.vector.tensor_scalar_add(
                    dst[0:D, o:o + n], ps[0:D, 0:n], bias_ap)

        return [lambda: sub(0), lambda: sub(1)]

    # ---- attention helpers ----------------------------------------------
    sst = {}
    pms = {}

    def spair(h, pr, cs=(0, 1)):
        for c in cs:
            for j in pr:
                if (h, j) not in sst:
                    sst[(h, j)] = ps_mm.tile([P, HI], F32, tag="ps_mm",
                                             name=f"ssT_{h}_{j}")
                nc.tensor.matmul(
                    sst[(h, j)][:, c * NC:(c + 1) * NC],
                    kT[:, j * P:(j + 1) * P],
                    qT[:, h * HI + c * NC:h * HI + (c + 1) * NC],
                    start=True, stop=True,
                )
        if 1 in cs:
            for j in pr:
                p = ppool.tile([P, HI], FP16, tag="pm", name=f"pm_{h}_{j}")
                nc.scalar.activation(p[:], sst[(h, j)][:],
                                     mybir.ActivationFunctionType.Exp,
                                     bias=mb[:, j:j + 1], scale=float(SCALE))
                pms[(h, j)] = p

    def sone(h, j, cs=(0, 1)):
        spair(h, (j,), cs=cs)

    xt = [None] * nj

    def x_group(js):
        for j in js:
            pst = ps_sm.tile([P, D + 1], FP16, tag="ps_sm", name=f"psx{j}")
            nc.tensor.transpose(pst[:], vT65[:, j * P:(j + 1) * P],
                                ident[0:D + 1, 0:D + 1])
            x = xpool.tile([P, D + 1], FP16, tag="x", name=f"x{j}")
            nc.vector.tensor_copy(x[:], pst[:])
            xt[j] = x

    def av_h0(num0, js):
        for j in js:
            for c in range(HI // NC):
                nc.tensor.matmul(
                    num0[:, c * NC:(c + 1) * NC],
                    xt[j][:],
                    pms[(0, j)][:, c * NC:(c + 1) * NC],
                    start=(j == 0), stop=(j == nj - 1),
                )

    # ---- emission --------------------------------------------------------
    def pq(i):
        o, n = QPIECES[i]
        proj_piece(qT, wq, bq, qs[i][:], o, n)

    def pk(i):
        o, n = kp[i]
        proj_piece(kT, wk, bk, ks[i][:], o, n)

    # num0 is allocated first so warm-up matmuls can target its PSUM; the
    # real AV h0 accumulation later resets it via start=True.
    num0 = ps_acc.tile([D + 1, HI], F32, tag="num", name="num0")

    # front: q half0 chunk0 + first key piece -> first score chunks;
    # q1 -> first exps; then the h0 per-j chain with k/q pieces between
    pq(0)
    pk(0)
    sone(0, 0, cs=(0,))
    if nj > 1:
        sone(0, 1, cs=(0,))
    pq(1)
    sone(0, 0, cs=(1,))
    if nj > 1:
        sone(0, 1, cs=(1,))
    for j in range(2, min(4, nj)):
        sone(0, j)
    if len(kp) > 1:
        pk(1)
    for j in range(4, min(6, nj)):
        sone(0, j)
    pq(2)
    for j in range(6, min(8, nj)):
        sone(0, j)
    for i in range(2, len(kp)):
        pk(i)
    pq(3)
    for j in range(8, nj):
        sone(0, j)

    # back half. The ssT ring (2 slots) means any h1 score matmul waits
    # deep into the exp chain; since the PE runs in order, ALL v fills /
    # x transposes / AV h0 / the h0 store must be emitted BEFORE the
    # first h1 score matmul. The h1 chain then interleaves with AV h1
    # j-steps so each lands right after the exp it needs.
    for i, (o, n) in enumerate(vp):
        for it in proj_items(vT65, wv, bv, vs[i][:], o, n):
            it()
    jsets = [list(range(o // P, (o + n) // P)) for (o, n) in vp]
    for js in jsets:
        x_group(js)
        av_h0(num0, js)
    nsb0 = fin.tile([D + 1, HI], FP16, tag="nsb0")
    nc.vector.tensor_copy(nsb0[:], num0[:])
    nc.sync.dma_start(out=out_d[0:D + 1, :], in_=nsb0[:])

    numc = [ps_sm.tile([D + 1, NC], F32, tag="ps_sm", name=f"num1c{c}")
            for c in range(HI // NC)]
    nsb1 = [fin.tile([D + 1, NC], FP16, tag=f"nsb1{c}", name=f"nsb1{c}")
            for c in range(HI // NC)]

    def av_h1(j):
        for c in range(HI // NC):
            nc.tensor.matmul(
                numc[c][:],
                xt[j][:],
                pms[(1, j)][:, c * NC:(c + 1) * NC],
                start=(j == 0), stop=(j == nj - 1),
            )
            if j == nj - 1:
                if c == 0:
                    nc.scalar.activation(
                        nsb1[c][:], numc[c][:],
                        mybir.ActivationFunctionType.Copy)
                else:
                    nc.vector.tensor_copy(nsb1[c][:], numc[c][:])
                nc.sync.dma_start(
                    out=out_d[D + 1:2 * (D + 1), c * NC:(c + 1) * NC],
                    in_=nsb1[c][:])

    sone(1, 0)
    if nj > 1:
        sone(1, 1)
    for j in range(2, nj):
        sone(1, j)
        av_h1(j - 2)
    av_h1(nj - 2) if nj >= 2 else None
    av_h1(nj - 1)


_COMPILED = {}


def _get_compiled(sk2: int):
    if sk2 not in _COMPILED:
        nj = sk2 // P
        kp = _kpieces(sk2)
        vp = _chunks(sk2, NC)
        nc = bacc.Bacc("TRN2", target_bir_lowering=False, debug=False,
                       num_devices=N_CORES)

        def din(name, shape, dt=FP16):
            return nc.dram_tensor(name, shape, dt, kind="ExternalInput").ap()

        ins = {"c16": din("c16", [P, 3 * NE * D]),
               "c32": din("c32", [P, nj + 3], F32)}
        for i, (o, n) in enumerate(QPIECES):
            ins[f"q{i}"] = din(f"q{i}", [P, NE * n])
        for i, (o, n) in enumerate(kp):
            ins[f"k{i}"] = din(f"k{i}", [P, NE * n])
        for i, (o, n) in enumerate(vp):
            ins[f"v{i}"] = din(f"v{i}", [P, NE * n])
        out_d = nc.dram_tensor("out", [NH * (D + 1), HI], FP16,
                               kind="ExternalOutput").ap()
        with tile.TileContext(nc) as tc:
            with ExitStack() as ctx:
                _build(tc, ins, out_d, ctx, sk2)
        nc.compile()
        _COMPILED[sk2] = nc
    return _COMPILED[sk2]


def _blob(x16, lo, hi):
    """[S', E] fp16 row-slice -> staging blob [P, NE*(hi-lo)] laid out as
    [partition, e-block, col]."""
    return np.ascontiguousarray(
        x16[lo:hi].reshape(hi - lo, NE, P).transpose(2, 1, 0)
    ).reshape(P, -1)


LAST_RESULTS = None


def kernel(query, key, value, query_mask, key_mask, Wq, bq, Wk, bk, Wv, bv):
    global LAST_RESULTS
    query = np.asarray(query, dtype=np.float32)
    key = np.asarray(key, dtype=np.float32)
    value = np.asarray(value, dtype=np.float32)
    key_mask = np.asarray(key_mask)

    # compact masked keys away (they contribute exactly zero)
    keeps = [np.nonzero(key_mask[c] != 0)[0] for c in range(N_CORES)]
    nk_max = max(len(kps) for kps in keeps)
    sk2 = max(P, int(np.ceil(nk_max / P)) * P)
    sk2 = min(sk2, S)
    nj = sk2 // P
    kp = _kpieces(sk2)
    vp = _chunks(sk2, NC)

    w16 = np.concatenate(
        [np.asarray(w, np.float32).astype(np.float16)
         .reshape(D, NE, P).transpose(2, 1, 0).reshape(P, NE * D)
         for w in (Wq, Wk, Wv)], axis=1)
    c32 = np.zeros((P, nj + 3), np.float32)
    for i, b in enumerate((bq, bk, bv)):
        c32[0:D, nj + i] = np.asarray(b, np.float32).reshape(D)

    in_maps = []
    for c in range(N_CORES):
        kps = keeps[c]
        nk = len(kps)
        q16 = query[c].astype(np.float16)
        kc = np.zeros((sk2, E), np.float16)
        vc = np.zeros((sk2, E), np.float16)
        kc[0:nk] = key[c][kps].astype(np.float16)
        vc[0:nk] = value[c][kps].astype(np.float16)
        c32c = c32.copy()
        mbias = np.full(sk2, np.float32(MASK_NEG))
        mbias[0:nk] = 0.0
        c32c[:, 0:nj] = mbias.reshape(nj, P).T
        im = {"c16": w16, "c32": np.ascontiguousarray(c32c)}
        for i, (o, n) in enumerate(QPIECES):
            im[f"q{i}"] = _blob(q16, o, o + n)
        for i, (o, n) in enumerate(kp):
            im[f"k{i}"] = _blob(kc, o, o + n)
        for i, (o, n) in enumerate(vp):
            im[f"v{i}"] = _blob(vc, o, o + n)
        in_maps.append(im)

    nc = _get_compiled(sk2)
    res = run_bass_kernel_spmd(nc, in_maps, core_ids=list(range(N_CORES)))
    LAST_RESULTS = res

    out = np.empty((N_CORES, S, D), np.float32)
    for c in range(N_CORES):
        o = np.asarray(res.results[c]["out"]).astype(np.float32)
        for h in range(NH):
            nh = o[h * (D + 1):(h + 1) * (D + 1)]
            out[c, h * HI:(h + 1) * HI] = (nh[0:D] / nh[D:D + 1]).T
    return out
